# revision 18
# baseline (speedup 1.0000x reference)
"""Trainium2 Bass kernel for nn_ConsolidationModel.

Mathematical reduction (verified bit-exact against the reference scan):
the scan's control flow is data-independent (count depends only on t).
Consolidation fires at t=15/31/47, but between consecutive firings the
8-slot FIFO receives 4 appends + 12 shift-appends, which evicts every
consolidated row before the next firing — and after the last firing
(t=47) there are 4 appends + 11 shifts, so at t=62 the buffer holds
exactly the embeddings of tokens 55..62 with count=8.  The model output
is therefore:

    mem  = mean_p embed[seqs[:, 55+p]]          (p = 0..7)
    h    = concat([embed[query_tok], mem], -1)  (B, 128)
    out  = relu(h @ r1_w.T + r1_b) @ r2_w.T + r2_b

Device algorithm (per core, batch shard of 256 rows; vocab/hidden on
partitions, batch on the free axis — 11 instructions total):

  wide (64, 2304) i16 <- ONE DMA with a stride-0 partition AP: the 9
                         token rows (8 tail positions + query)
                         broadcast across the 64 vocab partitions
  m8    = (wide[:, :2048] == iota)  bf16 one-hot masks        (1 DVE op)
  histT = bf16 add-tree over the 8 position masks             (3 DVE ops)
  qT    = (wide[:, 2048:] == iota)  f32                       (1 DVE op)
  hidT  = [B; A]^T @ [histT; qT]   ONE K=128 matmul, where
          A = embed @ r1_w[:, :64].T, B = embed @ r1_w[:, 64:].T / 8
          are folded on the host (data-independent weight prep)
  hid   = relu(hidT + r1_b)        one 2-op tensor_scalar
  logT  = r2_w.T^T @ hid           K=64 matmul
  out   = logT + r2_b              tensor_scalar add (PSUM -> SBUF)
  -> DMA out (64, 256); the host transposes each shard back to
     (256, 64) while gathering the 8 shards.

Sharding: pure data parallel over batch across 8 cores; parameters
replicated.
"""

import numpy as np

N_CORES = 8
B = 2048           # full batch
BS = B // N_CORES  # 256 per-core batch shard
H = 64             # hidden dim
V = 64             # vocab
TAIL_LO, TAIL_HI = 55, 63  # token positions that survive in the buffer
NPOS = TAIL_HI - TAIL_LO   # 8

_compiled_nc = None


def _build_program():
    import concourse.bacc as bacc
    import concourse.mybir as mybir
    from concourse import tile

    f32 = mybir.dt.float32
    bf16 = mybir.dt.bfloat16
    u8 = mybir.dt.uint8
    eq = mybir.AluOpType.is_equal
    add = mybir.AluOpType.add
    mx = mybir.AluOpType.max

    nc = bacc.Bacc("TRN2", target_bir_lowering=False, debug=False,
                   num_devices=N_CORES)

    i16 = mybir.dt.int16
    toks_d = nc.declare_dram_parameter("toks", [1, 9 * BS], i16, isOutput=False)
    cst_d = nc.declare_dram_parameter("cst", [2 * H, 131], f32, isOutput=False)
    out_d = nc.declare_dram_parameter("logT", [V, BS], f32, isOutput=True)

    with tile.TileContext(nc) as tc:
        with (
            tc.tile_pool(name="sb", bufs=1) as pool,
            tc.tile_pool(name="ps", bufs=1, space="PSUM") as pp,
        ):
            # iota column generated on-device: no DMA dependency for the eqs
            iota_t = pool.tile([V, 1], f32)
            nc.gpsimd.iota(iota_t[:], pattern=[[0, 1]], base=0,
                           channel_multiplier=1,
                           allow_small_or_imprecise_dtypes=True)
            iota = iota_t[:, 0:1]

            # token broadcast: stride-0 partition AP, int16 for the DVE
            # 2x mode
            wide = pool.tile([V, 9 * BS], i16)
            nc.sync.dma_start(wide[:], toks_d[:].to_broadcast((V, 9 * BS)))
            cst = pool.tile([2 * H, 131], f32)
            nc.scalar.dma_start(cst[:], cst_d[:])
            r1b = cst[0:H, 128:129]
            r2b = cst[0:V, 129:130]

            # one-hot masks + histogram (bf16 add tree; counts <= 8 exact)
            hq = pool.tile([2 * H, BS], f32)   # rows 0:64 histT, 64:128 qT
            m8 = pool.tile([V, NPOS * BS], bf16)
            nc.vector.tensor_scalar(m8[:], wide[:, 0:NPOS * BS], iota, None, eq)
            s2 = pool.tile([V, 4 * BS], bf16)
            nc.vector.tensor_add(s2[:], m8[:, 0:4 * BS], m8[:, 4 * BS:8 * BS])
            s4 = pool.tile([V, 2 * BS], bf16)
            nc.vector.tensor_add(s4[:], s2[:, 0:2 * BS], s2[:, 2 * BS:4 * BS])
            nc.vector.tensor_add(hq[0:V, :], s4[:, 0:BS], s4[:, BS:2 * BS])
            nc.vector.tensor_scalar(hq[V:2 * V, :], wide[:, NPOS * BS:9 * BS], iota, None, eq)

            # hidT = B^T @ histT + A^T @ qT   (single K=128 matmul)
            hidT_ps = pp.tile([H, BS], f32, tag="hid")
            nc.tensor.matmul(hidT_ps[:], cst[:, 0:64], hq[:], start=True, stop=True)
            # hid = relu(hidT + r1_b)   (scalar engine, off the DVE)
            hid = pool.tile([H, BS], f32)
            nc.scalar.activation(hid[:], hidT_ps[:],
                                 mybir.ActivationFunctionType.Relu,
                                 bias=r1b, scale=1.0)

            # logT = r2wT^T @ hid ; + r2_b on the PSUM->SBUF move
            logT_ps = pp.tile([V, BS], f32, tag="log")
            nc.tensor.matmul(logT_ps[:], cst[0:H, 64:128], hid[:], start=True, stop=True)
            logT_sb = pool.tile([V, BS], f32)
            nc.vector.tensor_scalar(logT_sb[:], logT_ps[:], r2b, None, add)
            nc.sync.dma_start(out_d[:], logT_sb[:])

    nc.compile()
    return nc


def _prep_in_maps(inputs):
    embed = np.asarray(inputs["embed"], dtype=np.float32)[:V]      # (64, 64)
    r1_w = np.asarray(inputs["r1_w"], dtype=np.float32)            # (64, 128)
    r1_b = np.asarray(inputs["r1_b"], dtype=np.float32)            # (64,)
    r2_w = np.asarray(inputs["r2_w"], dtype=np.float32)            # (64, 64)
    r2_b = np.asarray(inputs["r2_b"], dtype=np.float32)            # (64,)
    seqs = np.asarray(inputs["seqs"])                              # (B, 64) int
    query = np.asarray(inputs["query_tok"])                        # (B,) int

    A = embed @ r1_w[:, :H].T                                      # (64v, 64h)
    Bm = (embed @ r1_w[:, H:].T) * np.float32(1.0 / NPOS)          # (64v, 64h)
    cst = np.zeros((2 * H, 131), np.float32)
    cst[0:V, 0:64] = Bm
    cst[V:2 * V, 0:64] = A
    cst[0:H, 64:128] = r2_w.T
    cst[0:H, 128] = r1_b
    cst[0:V, 129] = r2_b
    cst[0:V, 130] = np.arange(V, dtype=np.float32)

    # token rows, position-major, then regrouped into the two batch
    # halves (64 | 192) the kernel pipelines over
    toks = np.empty((N_CORES, 9, BS), np.int16)
    toks[:, :NPOS, :] = (
        seqs[:, TAIL_LO:TAIL_HI].astype(np.int16).reshape(N_CORES, BS, NPOS)
        .transpose(0, 2, 1))
    toks[:, NPOS, :] = query.astype(np.int16).reshape(N_CORES, BS)

    return [
        {"toks": toks[c].reshape(1, 9 * BS), "cst": cst}
        for c in range(N_CORES)
    ]


def kernel(**inputs):
    global _compiled_nc
    from concourse.bass_utils import run_bass_kernel_spmd

    in_maps = _prep_in_maps(inputs)
    if _compiled_nc is None:
        _compiled_nc = _build_program()
    res = run_bass_kernel_spmd(_compiled_nc, in_maps, list(range(N_CORES)))
    out = np.empty((B, V), np.float32)
    for c in range(N_CORES):
        out[c * BS:(c + 1) * BS] = res.results[c]["logT"].T
    return out


if __name__ == "__main__":
    rng = np.random.default_rng(0)
    demo = {
        "embed": rng.standard_normal((V + 2, H)).astype(np.float32),
        "r1_w": rng.standard_normal((H, 2 * H)).astype(np.float32) * 0.05,
        "r1_b": rng.standard_normal(H).astype(np.float32) * 0.02,
        "r2_w": rng.standard_normal((V, H)).astype(np.float32) * 0.05,
        "r2_b": rng.standard_normal(V).astype(np.float32) * 0.02,
        "seqs": rng.integers(0, V, (B, 64)),
        "query_tok": rng.integers(0, V, (B,)),
    }
    out = kernel(**demo)
    tail = demo["embed"][demo["seqs"][:, TAIL_LO:TAIL_HI]]
    mem = tail.sum(1) / NPOS
    h = np.concatenate([demo["embed"][demo["query_tok"]], mem], -1)
    exp = np.maximum(h @ demo["r1_w"].T + demo["r1_b"], 0) @ demo["r2_w"].T + demo["r2_b"]
    err = np.abs(out - exp).max() / np.abs(exp).max()
    print("self-check rel err:", err)
